# revision 2
# baseline (speedup 1.0000x reference)
"""GAT message-passing kernel for Trainium2, 8 NeuronCores — v3.

For each head h:
    Wh   = x @ W[h]                                  [B,N,F]
    e    = leaky_relu((Wh@a_src)[:,:,None] + (Wh@a_dst)[:,None,:], 0.2)
    att  = exp(where(adj>0, e, -9e15)) * big_w        [B,N,N]
    att /= clip(sum(att, axis=1), 1e-12)              (column L1 norm)
    out_h = elu(att @ Wh)

big_w is bipartite: att has only two 1024x1024 nonzero blocks:
    A: (i<U, j>=U) = weights.T ; B: (i>=U, j<U) = weights.

Sharding: core c -> (b = c//4, h = c%4). Uniform SPMD, no collectives.

Design (v3):
  - Host staging reshapes everything partition-major so every DMA is a
    contiguous per-partition stream: x as xT [FIN,N]; weights as wmT and
    adj blocks pre-tiled to [128, 8*1024] ("(t p) v -> p (t v)").
    A single weights layout wmT serves BOTH att blocks:
      att_A[i, U+v]  = e_A * wmT[i,v] * adjA[i,v]     (natural layout)
      att_B^T[u,U+v] = e_B * wmT[u,v] * adjTB[u,v]    (transposed layout)
  - adj/wmT load via SWDGE (gpsimd) DMA with dtype cast to bf16 in the
    DMA datapath; contiguous layout keeps the Q7 descriptor count tiny.
  - Block B transposed: STT e*adjw with fused accum_out -> column denoms
    free. Block A natural: TT at 2x bf16 rate; denominators via 64 tiny
    PE ones-matmuls accumulating straight into a [128,8] PSUM tile; att_A
    xbar-DMA-transposed (bf16) into lhsT layout.
  - elu(y) = max(y,0) + min(exp(y),1) - 1 (exact), 1 ACT + 2 DVE ops from
    PSUM.
  - Emission interleaves block-B epilogue (out matmuls + elu) into the
    block-A loop so ACT (the bottleneck engine, ~37us) never idles.
"""

import threading
import numpy as np

B, N, FIN, F, H, U = 2, 2048, 128, 128, 4, 1024
V = N - U
P = 128
JT = U // P    # 8 tiles per block axis
ALPHA = 0.2

TRACE = False          # set by test.py for profiling runs
LAST_EXEC_NS = None    # exec_time_ns of the last traced run
_BUILD_LOCK = threading.Lock()
_CACHE = {}

CHUNK = 2      # jt tiles per SWDGE dma chunk (2 -> 1MB int32 chunks)
NCHUNK = JT // CHUNK


def _build_program():
    from concourse import bacc
    import concourse.mybir as mybir
    import concourse.tile as tile

    dt = mybir.dt
    Alu = mybir.AluOpType
    Act = mybir.ActivationFunctionType

    nc = bacc.Bacc("TRN2", target_bir_lowering=False, debug=False, num_devices=8)

    # all big inputs pre-tiled on host to [P, JT*V] partition-major
    xbT = nc.dram_tensor("xbT", [FIN, N], dt.float32, kind="ExternalInput")
    wmT = nc.dram_tensor("wmT", [P, JT * V], dt.float32, kind="ExternalInput")
    adjA = nc.dram_tensor("adjA", [P, JT * V], dt.int32, kind="ExternalInput")
    adjTB = nc.dram_tensor("adjTB", [P, JT * V], dt.int32, kind="ExternalInput")
    whp = nc.dram_tensor("whp", [FIN, F], dt.float32, kind="ExternalInput")
    av2 = nc.dram_tensor("av2", [F, 2], dt.float32, kind="ExternalInput")
    outh = nc.dram_tensor("outh", [N, F], dt.float32, kind="ExternalOutput")

    wmT_r = wmT.rearrange("p (t v) -> p t v", v=V)
    adjA_r = adjA.rearrange("p (t v) -> p t v", v=V)
    adjTB_r = adjTB.rearrange("p (t v) -> p t v", v=V)
    outh_r = outh.rearrange("(t p) f -> p t f", p=P)

    with tile.TileContext(nc) as tc:
        with (
            tc.tile_pool(name="persist", bufs=1) as persist,
            tc.tile_pool(name="lr", bufs=2) as lr_pool,
            tc.tile_pool(name="ee", bufs=2) as e_pool,
            tc.tile_pool(name="adjw", bufs=2) as adjw_pool,
            tc.tile_pool(name="eo", bufs=2) as eo_pool,
            tc.tile_pool(name="ps_mm", bufs=2, space="PSUM") as ps_mm,
            tc.tile_pool(name="ps_cols", bufs=1, space="PSUM") as ps_cols,
            tc.tile_pool(name="ps_den", bufs=1, space="PSUM") as ps_den,
            tc.tile_pool(name="ps_o", bufs=3, space="PSUM") as ps_o,
        ):
            # ---------------- DMA kickoff.
            # SP HWDGE ring: small/urgent tensors (xT gates all ACT work).
            w_f = persist.tile([P, F], dt.float32)
            nc.scalar.dma_start(out=w_f, in_=whp[:, :])
            a_f = persist.tile([P, 2], dt.float32)
            nc.scalar.dma_start(out=a_f, in_=av2[:, :])
            x_f = persist.tile([P, N], dt.float32, tag="xout")
            nc.sync.dma_start(out=x_f, in_=xbT[:, :])

            # SWDGE (gpsimd): big mask/weight tensors, bf16 cast in-DMA.
            # Order: [wmT, adjTB] interleaved (block B first), adjA last.
            wmT_b = persist.tile([P, JT, V], dt.bfloat16)
            adjB_b = persist.tile([P, JT, V], dt.bfloat16)
            adjA_b = persist.tile([P, JT, V], dt.bfloat16)
            # gate the big SWDGE streams behind xT's arrival so the small
            # urgent loads get exclusive HBM bandwidth first (the SDMA
            # round-robin would otherwise starve them ~5x)
            gate = persist.tile([P, 1], dt.float32)
            nc.gpsimd.tensor_copy(gate, x_f[:, 0:1])
            for c in range(2):
                s = slice(c * 4, (c + 1) * 4)
                nc.gpsimd.dma_start(out=wmT_b[:, s, :], in_=wmT_r[:, s, :])
                nc.gpsimd.dma_start(out=adjA_b[:, s, :], in_=adjA_r[:, s, :])
            for c in range(2):
                s = slice(c * 4, (c + 1) * 4)
                nc.gpsimd.dma_start(out=adjB_b[:, s, :], in_=adjTB_r[:, s, :])

            # ---------------- phase 0: Wh^T, scores
            xT = persist.tile([P, N], dt.float32r)
            nc.vector.tensor_copy(xT, x_f)
            w_sb = persist.tile([P, F], dt.float32r)
            nc.vector.tensor_copy(w_sb, w_f)
            a_r = persist.tile([P, 2], dt.float32r)
            nc.vector.tensor_copy(a_r, a_f)

            whT = persist.tile([P, N], dt.float32r, tag="slotB")  # [f, n]
            for q in range(4):
                wt_ps = ps_mm.tile([P, 512], dt.float32, tag="mm")
                nc.tensor.matmul(
                    wt_ps, w_sb, xT[:, q * 512 : (q + 1) * 512], start=True, stop=True
                )
                nc.vector.tensor_copy(whT[:, q * 512 : (q + 1) * 512], wt_ps)

            # wh rows [n, f] in bf16 via xbar transpose of whT (for whs)
            whTb = persist.tile([P, N], dt.bfloat16)
            nc.scalar.copy(whTb, whT)
            whb = persist.tile([P, 2 * JT, F], dt.bfloat16)  # [n-part, nt, f]
            nc.sync.dma_start(out=whb[:, 0:JT, :], in_=whTb[:, 0:U], transpose=True)
            nc.sync.dma_start(out=whb[:, JT : 2 * JT, :], in_=whTb[:, U:N], transpose=True)

            # score rows (upper halves; lower-half scores arrive as biases)
            s_hi = persist.tile([1, V], dt.float32r)
            d_hi = persist.tile([1, V], dt.float32r)
            for q in range(2):
                sl = slice(U + q * 512, U + (q + 1) * 512)
                s_ps = ps_mm.tile([1, 512], dt.float32, tag="mm")
                nc.tensor.matmul(s_ps, a_r[:, 0:1], whT[:, sl], start=True, stop=True)
                nc.vector.tensor_copy(s_hi[:, q * 512 : (q + 1) * 512], s_ps)
                d_ps = ps_mm.tile([1, 512], dt.float32, tag="mm")
                nc.tensor.matmul(d_ps, a_r[:, 1:2], whT[:, sl], start=True, stop=True)
                nc.vector.tensor_copy(d_hi[:, q * 512 : (q + 1) * 512], d_ps)

            # score cols: sd_cols[p, nt, 0] = s[nt*128+p], [.,.,1] = d[...]
            sdc_ps = ps_cols.tile([P, 2 * JT, 2], dt.float32)
            for t in range(2 * JT):
                nc.tensor.matmul(
                    sdc_ps[:, t, :],
                    whT[:, t * P : (t + 1) * P],
                    a_r,
                    start=True,
                    stop=True,
                )
            sd_cols = persist.tile([P, 2 * JT, 2], dt.float32)
            nc.vector.tensor_copy(sd_cols, sdc_ps)

            ones_b = persist.tile([P, 1], dt.bfloat16)
            nc.vector.memset(ones_b, 1.0)

            # materialized row-broadcasts via PE ones-outer-product (the
            # gpsimd path would queue behind the SWDGE issues for ~40us)
            ones_f = persist.tile([1, P], dt.float32)
            nc.vector.memset(ones_f, 1.0)
            ones_row = persist.tile([1, P], dt.float32r)
            nc.vector.tensor_copy(ones_row, ones_f)
            s_hi_bc = persist.tile([P, V], dt.float32)  # block B free axis
            d_hi_bc = persist.tile([P, V], dt.float32)  # block A free axis
            for q in range(2):
                sl = slice(q * 512, (q + 1) * 512)
                bc_ps = ps_mm.tile([P, 512], dt.float32, tag="mm")
                nc.tensor.matmul(bc_ps, ones_row, s_hi[:, sl], start=True, stop=True)
                nc.vector.tensor_copy(s_hi_bc[:, sl], bc_ps)
                bc_ps = ps_mm.tile([P, 512], dt.float32, tag="mm")
                nc.tensor.matmul(bc_ps, ones_row, d_hi[:, sl], start=True, stop=True)
                nc.vector.tensor_copy(d_hi_bc[:, sl], bc_ps)

            # ---------------- compute. Block A (natural layout, den via PE,
            # xbar transpose) first so its long epilogue (den matmuls, recA,
            # whs_hi, out_A) hides inside block B's main loop; block B
            # (fused STT denominators, near-zero epilogue) last. ACT ops are
            # quad-batched (one exp per 4 tiles, one elu-exp per 4 output
            # tiles) to amortize the per-op fixed cost on the bottleneck
            # engine.
            attA_n = persist.tile([P, JT, V], dt.bfloat16)   # natural layout
            attT_A = persist.tile([P, JT, V], dt.bfloat16)   # lhsT layout
            attT_B = persist.tile([P, JT, V], dt.bfloat16, tag="slotB")
            denB = persist.tile([P, JT], dt.float32)
            den_ps = ps_den.tile([P, JT], dt.float32)
            out_sb = persist.tile([P, 2 * JT, F], dt.float32, tag="xout")
            whs = persist.tile([P, 2 * JT, F], dt.bfloat16)
            recA = persist.tile([P, JT], dt.float32)
            recB = persist.tile([P, JT], dt.float32)

            def elu_quad(o_ps4, slot0):
                # elu(y) = max(y,0) + min(exp(y),1) - 1 over 4 output tiles
                eo = eo_pool.tile([P, 4 * F], dt.float32, tag="eo")
                nc.scalar.activation(eo, o_ps4, Act.Exp)
                em1 = eo_pool.tile([P, 4 * F], dt.float32, tag="em1")
                nc.vector.tensor_scalar(
                    out=em1, in0=eo, scalar1=1.0, scalar2=-1.0,
                    op0=Alu.min, op1=Alu.add,
                )
                nc.vector.scalar_tensor_tensor(
                    out=out_sb[:, slot0 : slot0 + 4, :], in0=o_ps4,
                    scalar=0.0, in1=em1, op0=Alu.max, op1=Alu.add,
                )

            def out_tile(att, whs_base, it, o_ps4, j):
                for k in range(JT):
                    nc.tensor.matmul(
                        o_ps4[:, j, :],
                        att[:, k, it * P : (it + 1) * P],
                        whs[:, whs_base + k, :],
                        start=(k == 0),
                        stop=(k == JT - 1),
                    )

            # block A main loop (2 quads)
            for q in range(2):
                lr4 = lr_pool.tile([P, 4, V], dt.float32, tag="lr")
                for j in range(4):
                    it = 4 * q + j
                    nc.scalar.activation(
                        lr4[:, j, :], d_hi_bc, Act.Prelu,
                        bias=sd_cols[:, it, 0:1], scale=1.0, alpha=ALPHA,
                    )
                e4 = e_pool.tile([P, 4, V], dt.bfloat16, tag="e")
                nc.scalar.activation(e4, lr4, Act.Exp)
                for j in range(4):
                    it = 4 * q + j
                    aw = adjw_pool.tile([P, V], dt.bfloat16, tag="aw")
                    nc.vector.tensor_tensor(
                        out=aw, in0=adjA_b[:, it, :], in1=wmT_b[:, it, :],
                        op=Alu.mult,
                    )
                    nc.vector.tensor_tensor(
                        out=attA_n[:, it, :], in0=e4[:, j, :], in1=aw, op=Alu.mult
                    )
                    nc.sync.dma_start(
                        out=attT_A[:, :, it * P : (it + 1) * P],
                        in_=attA_n[:, it, :],
                        transpose=True,
                    )

            # den_A[v]: den[v', k] = sum_i att_A[i, k*128+v'], one contiguous
            # accumulation group per k (groups sharing a PSUM bank must not
            # interleave).
            for k in range(JT):
                for it in range(JT):
                    nc.tensor.matmul(
                        den_ps[:, k : k + 1],
                        attA_n[:, it, k * P : (k + 1) * P],
                        ones_b,
                        start=(it == 0),
                        stop=(it == JT - 1),
                    )

            # block B main loop (2 quads), block A's epilogue woven in
            for q in range(2):
                lr4 = lr_pool.tile([P, 4, V], dt.float32, tag="lr")
                for j in range(4):
                    jt = 4 * q + j
                    nc.scalar.activation(
                        lr4[:, j, :], s_hi_bc, Act.Prelu,
                        bias=sd_cols[:, jt, 1:2], scale=1.0, alpha=ALPHA,
                    )
                e4 = e_pool.tile([P, 4, V], dt.bfloat16, tag="e")
                nc.scalar.activation(e4, lr4, Act.Exp)
                o_ps4 = None
                for j in range(4):
                    jt = 4 * q + j
                    aw = adjw_pool.tile([P, V], dt.bfloat16, tag="aw")
                    nc.vector.tensor_tensor(
                        out=aw, in0=adjB_b[:, jt, :], in1=wmT_b[:, jt, :],
                        op=Alu.mult,
                    )
                    nc.vector.scalar_tensor_tensor(
                        out=attT_B[:, jt, :], in0=e4[:, j, :], scalar=1.0,
                        in1=aw, op0=Alu.mult, op1=Alu.mult,
                        accum_out=denB[:, jt : jt + 1],
                    )
                    if q == 0 and j == 3:
                        # block A normalization (den_ps ready by now)
                        nc.vector.tensor_scalar(
                            out=recA, in0=den_ps, scalar1=1e-12, scalar2=None,
                            op0=Alu.max,
                        )
                        nc.vector.reciprocal(recA, recA)
                        for k in range(JT):
                            nc.vector.tensor_scalar(
                                out=whs[:, JT + k, :], in0=whb[:, JT + k, :],
                                scalar1=recA[:, k : k + 1], scalar2=None,
                                op0=Alu.mult,
                            )
                    if q == 1:
                        if j % 2 == 0:
                            o_ps4 = ps_o.tile([P, 4, F], dt.float32, tag="o")
                        for m in (2 * j, 2 * j + 1):
                            out_tile(attT_A, JT, m, o_ps4, m % 4)
                        if j % 2 == 1:
                            elu_quad(o_ps4, 4 * (j // 2))
            nc.sync.dma_start(out=outh_r[:, 0:JT, :], in_=out_sb[:, 0:JT, :])

            # block B epilogue
            nc.vector.tensor_scalar(
                out=recB, in0=denB, scalar1=1e-12, scalar2=None, op0=Alu.max
            )
            nc.vector.reciprocal(recB, recB)
            for jt in range(JT):
                nc.vector.tensor_scalar(
                    out=whs[:, jt, :], in0=whb[:, jt, :],
                    scalar1=recB[:, jt : jt + 1], scalar2=None, op0=Alu.mult,
                )
            for q in range(2):
                o_ps4 = ps_o.tile([P, 4, F], dt.float32, tag="o")
                for j in range(4):
                    out_tile(attT_B, 0, 4 * q + j, o_ps4, j)
                elu_quad(o_ps4, JT + 4 * q)
            nc.sync.dma_start(
                out=outh_r[:, JT : 2 * JT, :], in_=out_sb[:, JT : 2 * JT, :]
            )

    nc.compile()
    return nc


def _tile_pmajor(m):
    # [U, V] -> [P, JT*V]: row (t*128+p) -> partition p, free block t
    return np.ascontiguousarray(
        m.reshape(JT, P, V).transpose(1, 0, 2).reshape(P, JT * V)
    )


def kernel(x, weights, W, a, adj):
    global LAST_EXEC_NS
    from concourse.bass_utils import run_bass_kernel_spmd

    x = np.asarray(x, dtype=np.float32)
    weights = np.asarray(weights, dtype=np.float32)
    W = np.asarray(W, dtype=np.float32)
    a = np.asarray(a, dtype=np.int32 if False else np.float32)
    adj = np.asarray(adj, dtype=np.int32)

    with _BUILD_LOCK:
        if "nc" not in _CACHE:
            _CACHE["nc"] = _build_program()
    nc = _CACHE["nc"]

    in_maps = []
    for c in range(8):
        b, h = c // 4, c % 4
        in_maps.append(
            {
                "xbT": np.ascontiguousarray(x[b].T),
                "wmT": _tile_pmajor(weights[b].T),
                "adjA": _tile_pmajor(adj[b, :U, U:]),
                "adjTB": _tile_pmajor(np.ascontiguousarray(adj[b, U:, :U].T)),
                "whp": W[h],
                "av2": np.ascontiguousarray(a[h, :, 0].reshape(2, F).T),
            }
        )

    res = run_bass_kernel_spmd(nc, in_maps, core_ids=list(range(8)), trace=TRACE)
    if res.exec_time_ns is not None:
        LAST_EXEC_NS = res.exec_time_ns

    out = np.empty((B, N, H * F), dtype=np.float32)
    for c in range(8):
        b, h = c // 4, c % 4
        out[b, :, h * F : (h + 1) * F] = res.results[c]["outh"]
    return out
